# revision 64
# baseline (speedup 1.0000x reference)
"""Causal linear attention (elu+1 feature map) Trainium2 Bass kernel.

Full inputs q,k,v: [4, 2048, 12, 64] fp32 -> out [4, 2048, 12, 64] fp32.
Sharding: 48 (batch, head) pairs, 6 per core across 8 NeuronCores; each
core processes 3 "pair-pairs" (2 pairs stacked on the 128 partitions).

Math per (n,h) pair, per 128-chunk g (exact algebraic regrouping):
  phi(x) = elu(x)+1 = max(x+1, exp(min(x, 0)))   [identity: exp>=x+1 on x<0]
  S_g    = sum_{g'<=g} kf_chunk^T v_chunk        (PSUM kv + segmented scan)
  K1_g   = sum_{g'<=g} kf_chunk^T 1
  scT    = kfT^T qfT, masked to s<=c
  num    = qf @ S_{g-1} + scT^T @ v              (PSUM accumulate)
  z      = qf @ K1_{g-1} + scT^T @ 1
  out    = num / z

All matmuls in bf16 (1 PE cycle/row vs 4 for fp32). Inputs are converted
to bf16 and laid out on the host; output returns as bf16 and is upcast on
the host. Engine assignment balances DVE / Pool / ACT:
  DVE:  phi min + combine (ts 4x / tt 2x), reciprocal, normalize
  ACT:  exp, kn PSUM->SBUF copies
  Pool: segmented scans, masked score evacuation (stt)
  PE:   all matmuls + per-chunk transposes of phi(k) into [c,d] layout
"""

import json
import os

import numpy as np

# ---------------------------------------------------------------------------
# Workaround for walrus "Too many sync wait commands": cap waits per
# instruction at 1, hoisting overflow onto same-engine NoOps inserted
# immediately before (engines run their stream in order, so consecutive
# waits AND together identically).
# ---------------------------------------------------------------------------
_wsplit_counter = [0]


def _split_instruction_waits(inst):
    si = inst.get("sync_info")
    if not si:
        return []
    waits = si.get("on_wait") or []
    if len(waits) <= 1:
        return []
    si["on_wait"] = waits[-1:]
    nops = []
    for w in waits[:-1]:
        _wsplit_counter[0] += 1
        nops.append(
            {
                "debug": inst.get("debug", 0),
                "engine": inst["engine"],
                "ins": [],
                "name": f"I-wsplit-{_wsplit_counter[0]}",
                "opcode": "NoOp",
                "outs": [],
                "sync_info": {"on_update": [], "on_wait": [w]},
            }
        )
    return nops


def _fix_module_json(raw: bytes) -> bytes:
    m = json.loads(raw)
    changed = False
    for f in m.get("functions", []):
        for b in f.get("blocks", []):
            out = []
            for inst in b.get("instructions", []):
                nops = _split_instruction_waits(inst)
                if nops:
                    changed = True
                    out.extend(nops)
                out.append(inst)
            b["instructions"] = out
    return json.dumps(m).encode() if changed else raw


_patch_installed = [False]


def _install_bir_patch():
    if _patch_installed[0]:
        return
    _patch_installed[0] = True
    import concourse.bass as _bass

    _orig = _bass.Bass.to_json_bytes

    def _patched(self):
        return _fix_module_json(_orig(self))

    _bass.Bass.to_json_bytes = _patched


# ---------------------------------------------------------------------------
# Problem constants (hardcoded per contest contract)
# ---------------------------------------------------------------------------
B, L, H, D = 4, 2048, 12, 64
CHUNK = 128
G = L // CHUNK  # 16
N_CORES = 8
PAIRS = [(n, h) for n in range(B) for h in range(H)]  # 48
PER_CORE = len(PAIRS) // N_CORES  # 6
NPP = PER_CORE // 2  # 3 pair-pairs per core

# engine assignment knobs (tuned against TimelineSim).
# GPSIMD/Pool cannot access PSUM on TRN2 and only plain TensorScalar /
# TensorTensor ops are Pool-legal, so PSUM-evacuation passes (mask, norm,
# scans) run on DVE/ACT; Pool takes SBUF-side work (phi max-combine,
# affine_select mask stage 2).
MASK_MODE = {(0, 0): "two", (0, 1): "two", (1, 0): "two", (1, 1): "two"}
U_ON_POOL = False  # phi min-stage on Pool (tensor_scalar_min is Pool-legal)
KN_EVAC_ACT = (0,)  # pair indices whose kn copy runs on ACT instead of DVE
PP_COUNT = NPP  # pair-pairs to emit (timing experiments)
XP1_POOL = ("k",)  # tensors whose +1 add runs on Pool
OUT_ENG = lambda nc: nc.sync  # output DMA queue
INS_BUFS = 2
WORK_BUFS = 3
OUTSB_BUFS = 2
PHASE_ORDER = [(0, 0), (1, 0), (0, 1), (1, 1)]
OUT_PER_PHASE = True
LAST_MASK_DVE = True
LAST_N_DVE = 1
Z_FIRST = False
FIRST_MASK_DVE = False
# ablation switches (timing experiments only; wrong numerics when set)
SKIP = set()


def _build_bass():
    import concourse.bass as bass
    import concourse.tile as tile
    import concourse.mybir as mybir

    fp32 = mybir.dt.float32
    bf16 = mybir.dt.bfloat16
    AF = mybir.ActivationFunctionType
    ALU = mybir.AluOpType

    nc = bass.Bass()
    qt = nc.dram_tensor("qt", [NPP, 128, L], bf16, kind="ExternalInput")
    kt = nc.dram_tensor("kt", [NPP, 128, L], bf16, kind="ExternalInput")
    vt = nc.dram_tensor("vt", [NPP, 128, 2, G, D], bf16, kind="ExternalInput")
    mask = nc.dram_tensor("mask", [128, CHUNK], fp32, kind="ExternalInput")
    ident = nc.dram_tensor("ident", [128, D], bf16, kind="ExternalInput")
    on = nc.dram_tensor("on", [NPP, 128, 2, G, D], bf16, kind="ExternalOutput")

    with tile.TileContext(nc) as tc:
        with (
            tc.tile_pool(name="singles", bufs=1) as singles,
            tc.tile_pool(name="ins", bufs=INS_BUFS) as ins,
            tc.tile_pool(name="work", bufs=WORK_BUFS) as work,
            tc.tile_pool(name="ps_knt", bufs=1, space="PSUM") as ps_knt,
            tc.tile_pool(name="ps_kv", bufs=1, space="PSUM") as ps_kv,
            tc.tile_pool(name="ps_kz", bufs=1, space="PSUM") as ps_kz,
            tc.tile_pool(name="ps_sc", bufs=1, space="PSUM") as ps_sc,
            tc.tile_pool(name="ps_num", bufs=2, space="PSUM") as ps_num,
        ):
            maskb = singles.tile([128, CHUNK], fp32)
            nc.sync.dma_start(out=maskb[:], in_=mask[:])
            identT = singles.tile([128, D], bf16)
            nc.sync.dma_start(out=identT[:], in_=ident[:])
            onesb = singles.tile([128, 1], bf16)
            nc.vector.memset(onesb[:], 1.0)
            seg_main = singles.tile([128, D, 8], fp32)
            nc.vector.memset(seg_main[:], 1.0)
            nc.vector.memset(seg_main[:, :, 0:1], 0.0)
            seg16 = singles.tile([128, G], fp32)
            nc.vector.memset(seg16[:], 1.0)
            nc.vector.memset(seg16[:, 0:1], 0.0)

            st = {}

            def proA(pp):
                d = st.setdefault(pp, {})
                qr = ins.tile([128, L], bf16, tag="qr", name=f"qr{pp}")
                kr = ins.tile([128, L], bf16, tag="kr", name=f"kr{pp}")
                vv = ins.tile([128, 2, G, D], bf16, tag="vv", name=f"vv{pp}")
                nc.sync.dma_start(out=qr[:], in_=qt[pp])
                nc.scalar.dma_start(out=kr[:], in_=kt[pp])
                nc.sync.dma_start(out=vv[:], in_=vt[pp])
                d["qr"], d["kr"], d["vv"] = qr, kr, vv

            def proB(pp):
                d = st[pp]
                qr, kr, vv = d["qr"], d["kr"], d["vv"]
                uq = work.tile([128, L], bf16, tag="uq", name=f"uq{pp}")
                uk = work.tile([128, L], bf16, tag="uk", name=f"uk{pp}")
                ek = work.tile([128, L], bf16, tag="ek", name=f"ek{pp}")
                eq = work.tile([128, L], bf16, tag="eq", name=f"eq{pp}")
                if "phi" not in SKIP:
                    nc.vector.tensor_scalar_min(out=uk[:], in0=kr[:],
                                                scalar1=0.0)
                    nc.scalar.activation(out=ek[:], in_=uk[:], func=AF.Exp)
                    nc.vector.tensor_scalar_min(out=uq[:], in0=qr[:],
                                                scalar1=0.0)
                    nc.scalar.activation(out=eq[:], in_=uq[:], func=AF.Exp)
                kf = ins.tile([128, L], bf16, tag="kf", name=f"kf{pp}")
                qf = ins.tile([128, L], bf16, tag="qf", name=f"qf{pp}")
                d["kf"], d["qf"] = kf, qf
                for raw, ex, dst in ((kr, ek, kf), (qr, eq, qf)):
                    if "phi" in SKIP:
                        nc.vector.tensor_copy(out=dst[:], in_=raw[:])
                        continue
                    nc.vector.tensor_scalar_add(out=dst[:], in0=raw[:],
                                                scalar1=1.0)
                    nc.vector.tensor_max(out=dst[:], in0=dst[:], in1=ex[:])
                kns = []
                for s in (0, 1):
                    po = D * s
                    knp = ps_knt.tile([128, G, D], bf16, tag="knp",
                                      name=f"knp{pp}_{s}")
                    for g in range(G):
                        nc.tensor.transpose(
                            knp[:, g, :],
                            kf[po : po + D, g * CHUNK : (g + 1) * CHUNK],
                            identT[po : po + D, :],
                        )
                    kn = ins.tile([128, G, D], bf16, tag=f"kn{s}",
                                  name=f"kn{pp}_{s}")
                    if s in KN_EVAC_ACT:
                        nc.scalar.copy(out=kn[:], in_=knp[:])
                    else:
                        nc.vector.tensor_copy(out=kn[:], in_=knp[:])
                    kns.append(kn)
                kvp = ps_kv.tile([128, 2, D, 8], fp32, tag="kv",
                                 name=f"kv{pp}")
                kzp = ps_kz.tile([128, 48], fp32, tag="kz", name=f"kz{pp}")
                d["kzp"] = kzp
                for s in (0, 1):
                    po = D * s
                    for g in range(G):
                        h, gh = divmod(g, 8)
                        nc.tensor.matmul(
                            kvp[po : po + D, h, :, gh], kns[s][:, g, :],
                            vv[:, s, g, :],
                            start=(gh == 0), stop=(gh == 7),
                            skip_group_check=True)
                        nc.tensor.matmul(
                            kzp[po : po + D, g : g + 1], kns[s][:, g, :],
                            onesb[:],
                            start=(g == 0), stop=False, skip_group_check=True)
                kvs = work.tile([128, 2, D, 8], bf16, tag="kvs",
                                name=f"kvs{pp}")
                k1c = work.tile([128, G], bf16, tag="k1c", name=f"k1c{pp}")
                d["kvs"], d["k1c"] = kvs, k1c
                for h in (0, 1):
                    nc.vector.tensor_tensor_scan(
                        out=kvs[:, h].rearrange("p m g -> p (m g)"),
                        data0=seg_main[:].rearrange("p m g -> p (m g)"),
                        data1=kvp[:, h].rearrange("p m g -> p (m g)"),
                        initial=0.0, op0=ALU.mult, op1=ALU.add)
                nc.vector.tensor_add(
                    out=kvs[:, 1], in0=kvs[:, 1],
                    in1=kvs[:, 0, :, 7:8].broadcast_to([128, D, 8]))
                nc.vector.tensor_tensor_scan(
                    out=k1c[:], data0=seg16[:], data1=kzp[:, 0:G],
                    initial=0.0, op0=ALU.mult, op1=ALU.add)

            def phases(pp):
                d = st[pp]
                kf, qf = d["kf"], d["qf"]
                vv, kvs, k1c, kzp = d["vv"], d["kvs"], d["k1c"], d["kzp"]
                outsb = ins.tile([128, 2, G, D], bf16, tag="outsb",
                                 name=f"outsb{pp}", bufs=OUTSB_BUFS)
                for s, h in PHASE_ORDER:
                    if True:
                        po = D * s
                        zc = 16 + 16 * s
                        hg = slice(8 * h, 8 * h + 8)
                        nums = ps_num.tile([128, 8, D], fp32, tag="num",
                                           name=f"num{pp}_{s}_{h}")
                        scp = ps_sc.tile([128, 8, CHUNK], fp32, tag="sc",
                                         name=f"sc{pp}_{s}_{h}")
                        for i in range(8):
                            g = 8 * h + i
                            cs = slice(g * CHUNK, (g + 1) * CHUNK)
                            nc.tensor.matmul(
                                scp[:, i, :], kf[po : po + D, cs],
                                qf[po : po + D, cs],
                                start=True, stop=True, skip_group_check=True)
                        for i in range(8):
                            g = 8 * h + i
                            if g == 0:
                                continue
                            cs = slice(g * CHUNK, (g + 1) * CHUNK)
                            hp, ghp = divmod(g - 1, 8)
                            nc.tensor.matmul(
                                nums[:, i, :], qf[po : po + D, cs],
                                kvs[po : po + D, hp, :, ghp],
                                start=(i == 0 or g == 1), stop=False,
                                skip_group_check=True)
                            nc.tensor.matmul(
                                kzp[:, zc + g : zc + g + 1],
                                qf[po : po + D, cs],
                                k1c[po : po + D, g - 1 : g],
                                start=False, stop=False,
                                skip_group_check=True)
                        scb = work.tile([128, 8, CHUNK], bf16, tag="scb",
                                        name=f"scb{pp}_{s}_{h}")
                        mmode = MASK_MODE[(s, h)]
                        if LAST_MASK_DVE and pp == PP_COUNT - 1 \
                                and (s, h) in PHASE_ORDER[-LAST_N_DVE:]:
                            mmode = "dve"
                        if FIRST_MASK_DVE and pp == 0 \
                                and (s, h) == PHASE_ORDER[0]:
                            mmode = "dve"
                        if mmode == "two":
                            scf = work.tile([128, 8, CHUNK], bf16, tag="scf",
                                            name=f"scf{pp}_{s}_{h}")
                            nc.scalar.copy(out=scf[:], in_=scp[:])
                            nc.gpsimd.affine_select(
                                out=scb[:], in_=scf[:],
                                pattern=[[0, 8], [1, CHUNK]],
                                compare_op=ALU.is_ge, fill=0.0, base=0,
                                channel_multiplier=-1)
                        else:
                            mb = maskb[:].unsqueeze(1).broadcast_to(
                                [128, 8, CHUNK])
                            nc.vector.tensor_mul(out=scb[:], in0=scp[:],
                                                 in1=mb)
                        if Z_FIRST:
                            for i in range(8):
                                g = 8 * h + i
                                nc.tensor.matmul(
                                    kzp[:, zc + g : zc + g + 1], scb[:, i, :],
                                    onesb[:],
                                    start=False,
                                    stop=(s == 1 and g == G - 1),
                                    skip_group_check=True)
                            for i in range(8):
                                g = 8 * h + i
                                nc.tensor.matmul(
                                    nums[:, i, :], scb[:, i, :],
                                    vv[:, s, g, :],
                                    start=False, stop=(i == 7),
                                    skip_group_check=True)
                        else:
                            for i in range(8):
                                g = 8 * h + i
                                nc.tensor.matmul(
                                    nums[:, i, :], scb[:, i, :],
                                    vv[:, s, g, :],
                                    start=False, stop=(i == 7),
                                    skip_group_check=True)
                                nc.tensor.matmul(
                                    kzp[:, zc + g : zc + g + 1], scb[:, i, :],
                                    onesb[:],
                                    start=False,
                                    stop=(s == 1 and g == G - 1),
                                    skip_group_check=True)
                        r4 = work.tile([128, 8], fp32, tag="r4",
                                       name=f"r4{pp}_{s}_{h}")
                        nc.vector.reciprocal(
                            out=r4[:],
                            in_=kzp[:, zc + 8 * h : zc + 8 * h + 8])
                        rb = r4[:].unsqueeze(2).broadcast_to([128, 8, D])
                        nc.vector.tensor_mul(out=outsb[:, s, hg],
                                             in0=nums[:], in1=rb)
                        if OUT_PER_PHASE or h == 1:
                            osl = hg if OUT_PER_PHASE else slice(0, G)
                            OUT_ENG(nc).dma_start(out=on[pp, :, s, osl],
                                                  in_=outsb[:, s, osl])


            # hoist only DMA+min+Exp of pp+1 ahead of phases(pp): unblocks
            # the next pp's ACT-gated phi chain without delaying this pp's
            # DVE phase tail by more than the two cheap 4x-mode min ops.
            proA(0)
            proB(0)
            for pp in range(PP_COUNT):
                if pp + 1 < PP_COUNT:
                    proA(pp + 1)
                phases(pp)
                if pp + 1 < PP_COUNT:
                    proB(pp + 1)

    return nc


_cached = {}


def _prep_inputs(q, k, v):
    import ml_dtypes

    bf = ml_dtypes.bfloat16
    maskarr = np.ascontiguousarray(
        np.tril(np.ones((CHUNK, CHUNK), np.float32)).T
    )  # [s, c] : 1 if s<=c
    identarr = np.concatenate([np.eye(D, dtype=np.float32)] * 2, axis=0).astype(bf)
    in_maps = []
    for c in range(N_CORES):
        sel = PAIRS[c * PER_CORE : (c + 1) * PER_CORE]
        qtl = np.empty((NPP, 128, L), bf)
        ktl = np.empty((NPP, 128, L), bf)
        vtl = np.empty((NPP, 128, 2, G, D), bf)
        for j in range(NPP):
            for s in (0, 1):
                n, h = sel[2 * j + s]
                qtl[j, D * s : D * s + D] = q[n, :, h, :].T.astype(bf)
                ktl[j, D * s : D * s + D] = k[n, :, h, :].T.astype(bf)
                vtl[j, :, s] = (
                    v[n, :, h, :].reshape(G, CHUNK, D).transpose(1, 0, 2)
                ).astype(bf)
        in_maps.append(
            {"qt": qtl, "kt": ktl, "vt": vtl, "mask": maskarr, "ident": identarr}
        )
    return in_maps


def kernel(q: np.ndarray, k: np.ndarray, v: np.ndarray) -> np.ndarray:
    _install_bir_patch()
    from concourse.bass_utils import run_bass_kernel_spmd

    if "nc" not in _cached:
        _cached["nc"] = _build_bass()
    nc = _cached["nc"]

    in_maps = _prep_inputs(q, k, v)
    try:
        res = run_bass_kernel_spmd(nc, in_maps, core_ids=list(range(N_CORES)))
    except ModuleNotFoundError:
        # BASS_TRACE=1 with no axon NTFF hook in the container: retry untraced
        os.environ["BASS_NEVER_TRACE"] = "1"
        res = run_bass_kernel_spmd(nc, in_maps, core_ids=list(range(N_CORES)))
    _cached["last_result"] = res

    out = np.empty((B, L, H, D), np.float32)
    for c in range(N_CORES):
        sel = PAIRS[c * PER_CORE : (c + 1) * PER_CORE]
        o = res.results[c]["on"]  # [NPP, 128, 2, G, D] bf16
        for j in range(NPP):
            for s in (0, 1):
                n, h = sel[2 * j + s]
                # [c, G, D] -> [L, D]
                out[n, :, h, :] = (
                    o[j, :, s].astype(np.float32).transpose(1, 0, 2).reshape(L, D)
                )
    return out
